# revision 3
# baseline (speedup 1.0000x reference)
"""Llama4-style MoE (top-1 routing, E=8) + shared SwiGLU expert on 8 Trainium2 cores.

Strategy (expert-parallel + data-parallel shared, host dispatch/combine):
  - Host computes router logits / top-1 routing / sigmoid scaling (0.03% of FLOPs)
    and sorts tokens by expert — this is the "dispatch" step of the sharding.
  - Core e gets: the tokens routed to expert e (zero-padded to a uniform CAP,
    transposed to [H, CAP]), expert e's SwiGLU weights (pre-transposed on host),
    plus a 1/8 slice of all tokens and the replicated shared-expert weights.
  - Each core runs two SwiGLU MLPs (routed segment + shared segment) and the
    router-logits matmul for its shared slice. All matmuls in float32r
    (full fp32 storage, ~bf16-rate on the PE at free-dim >= 256).
  - Host "combine": routed and shared partial outputs are summed and scattered
    back to the original token order.

All device work is plain dense GEMMs with static shapes; the data-dependent part
(group sizes) only affects CAP, which is rounded to a multiple of 128 so the
compiled program is stable for a given input distribution.
"""

import numpy as np
from contextlib import ExitStack

import concourse.bacc as bacc
import concourse.tile as tile
from concourse import mybir
from concourse.bass_utils import run_bass_kernel_spmd

P = 128
H = 2048
I = 2048
E = 8
NCORES = 8

f32 = mybir.dt.float32
f32r = mybir.dt.float32r

_prog_cache: dict = {}


def _m_tiles(M, width=512):
    out = []
    m0 = 0
    while m0 < M:
        w = min(width, M - m0)
        out.append((m0, w))
        m0 += w
    return out


def _chunks(lst, n):
    return [lst[i:i + n] for i in range(0, len(lst), n)]


def _build_program(cap, S):
    nc = bacc.Bacc("TRN2", target_bir_lowering=False, debug=False,
                   num_devices=NCORES)

    xr = nc.declare_dram_parameter("xr", [H, cap], f32r, isOutput=False)
    xs = nc.declare_dram_parameter("xs", [H, S], f32r, isOutput=False)
    wg = nc.declare_dram_parameter("wg", [H, I], f32r, isOutput=False)
    wu = nc.declare_dram_parameter("wu", [H, I], f32r, isOutput=False)
    wd = nc.declare_dram_parameter("wd", [I, H], f32r, isOutput=False)
    sg = nc.declare_dram_parameter("sg", [H, I], f32r, isOutput=False)
    su = nc.declare_dram_parameter("su", [H, I], f32r, isOutput=False)
    sd = nc.declare_dram_parameter("sd", [I, H], f32r, isOutput=False)
    gwt = nc.declare_dram_parameter("gwt", [H, E], f32r, isOutput=False)
    outr = nc.declare_dram_parameter("outr", [cap, H], f32, isOutput=True)
    outs = nc.declare_dram_parameter("outs", [S, H], f32, isOutput=True)
    logt = nc.declare_dram_parameter("logt", [E, S], f32, isOutput=True)

    HC = H // P    # 16 contraction chunks
    IC = I // P    # 16 i chunks
    HF = H // 512  # 4 output free chunks (stage 2)

    with tile.TileContext(nc) as tc, ExitStack() as ctx:
        xpool = ctx.enter_context(tc.tile_pool(name="xp", bufs=1))
        w1pool = ctx.enter_context(tc.tile_pool(name="w1", bufs=2))
        hpool = ctx.enter_context(tc.tile_pool(name="hp", bufs=1))
        w2pool = ctx.enter_context(tc.tile_pool(name="w2", bufs=3))
        tpool = ctx.enter_context(tc.tile_pool(name="tp", bufs=2))
        opool = ctx.enter_context(tc.tile_pool(name="op", bufs=3))
        ps1 = ctx.enter_context(tc.tile_pool(name="ps1", bufs=2, space="PSUM"))
        ps2 = ctx.enter_context(tc.tile_pool(name="ps2", bufs=4, space="PSUM"))

        segs = [
            ("r", xr, cap, wg, wu, wd, outr),
            ("s", xs, S, sg, su, sd, outs),
        ]

        for sname, xdram, M, wgd, wud, wdd, odram in segs:
            xt = xpool.tile([P, HC, M], f32r, name=f"xt_{sname}",
                            tag=f"xt_{sname}")
            nc.sync.dma_start(
                out=xt,
                in_=xdram.ap().rearrange("(hc p) m -> p hc m", p=P))

            ht = hpool.tile([P, IC, M], f32r, name=f"ht_{sname}",
                            tag=f"ht_{sname}")

            # ---- stage 1: g = Wg^T x, u = Wu^T x, ht = silu(g) * u ----
            wgv = wgd.ap().rearrange("(hc p) i -> p hc i", p=P)
            wuv = wud.ap().rearrange("(hc p) i -> p hc i", p=P)
            for ig in range(IC):
                wgt = w1pool.tile([P, HC, P], f32r, name="wgt", tag="wgt")
                wut = w1pool.tile([P, HC, P], f32r, name="wut", tag="wut")
                nc.sync.dma_start(out=wgt, in_=wgv[:, :, ig * P:(ig + 1) * P])
                nc.sync.dma_start(out=wut, in_=wuv[:, :, ig * P:(ig + 1) * P])
                for (m0, mw) in _m_tiles(M):
                    pg = ps1.tile([P, 512], f32, name="pg", tag="pg")
                    pu = ps1.tile([P, 512], f32, name="pu", tag="pu")
                    for hc in range(HC):
                        nc.tensor.matmul(pg[:, :mw], lhsT=wgt[:, hc, :],
                                         rhs=xt[:, hc, m0:m0 + mw],
                                         start=(hc == 0), stop=(hc == HC - 1))
                    for hc in range(HC):
                        nc.tensor.matmul(pu[:, :mw], lhsT=wut[:, hc, :],
                                         rhs=xt[:, hc, m0:m0 + mw],
                                         start=(hc == 0), stop=(hc == HC - 1))
                    sil = tpool.tile([P, 512], f32, name="sil", tag="sil")
                    nc.scalar.activation(sil[:, :mw], pg[:, :mw],
                                         mybir.ActivationFunctionType.Silu)
                    nc.vector.tensor_mul(ht[:, ig, m0:m0 + mw], sil[:, :mw],
                                         pu[:, :mw])

            # ---- stage 2: out = ht^T @ Wd^T ----
            mp_tiles = _m_tiles(M, 128)
            for mp_chunk in _chunks(mp_tiles, 4):
                for hf in range(HF):
                    pds = []
                    for ci, (m0, mw) in enumerate(mp_chunk):
                        pds.append(ps2.tile([P, 512], f32, name=f"pd{ci}",
                                            tag="pd"))
                    for ic in range(IC):
                        wdt = w2pool.tile([P, 512], f32r, name="wdt", tag="wdt")
                        nc.sync.dma_start(
                            out=wdt,
                            in_=wdd.ap()[ic * P:(ic + 1) * P,
                                         hf * 512:(hf + 1) * 512])
                        for ci, (m0, mw) in enumerate(mp_chunk):
                            nc.tensor.matmul(
                                pds[ci][:mw, :],
                                lhsT=ht[:, ic, m0:m0 + mw],
                                rhs=wdt,
                                start=(ic == 0), stop=(ic == IC - 1))
                    for ci, (m0, mw) in enumerate(mp_chunk):
                        ot = opool.tile([P, 512], f32, name="ot", tag="ot")
                        nc.vector.tensor_copy(ot[:mw, :], pds[ci][:mw, :])
                        nc.sync.dma_start(
                            out=odram.ap()[m0:m0 + mw,
                                           hf * 512:(hf + 1) * 512],
                            in_=ot[:mw, :])

            if sname == "s":
                # ---- router logits for this core's shared token slice ----
                gwtile = xpool.tile([P, HC, E], f32r, name="gwtile",
                                    tag="gwtile")
                nc.sync.dma_start(
                    out=gwtile,
                    in_=gwt.ap().rearrange("(hc p) e -> p hc e", p=P))
                for (m0, mw) in _m_tiles(M):
                    pl = ps2.tile([E, 512], f32, name="pl", tag="pd")
                    for hc in range(HC):
                        nc.tensor.matmul(pl[:, :mw], lhsT=gwtile[:, hc, :],
                                         rhs=xt[:, hc, m0:m0 + mw],
                                         start=(hc == 0), stop=(hc == HC - 1))
                    lt = opool.tile([E, 512], f32, name="lt", tag="lt")
                    nc.vector.tensor_copy(lt[:, :mw], pl[:, :mw])
                    nc.sync.dma_start(out=logt.ap()[:, m0:m0 + mw],
                                      in_=lt[:, :mw])

    nc.compile()
    return nc


def _get_program(cap, S):
    key = (cap, S)
    if key not in _prog_cache:
        _prog_cache[key] = _build_program(cap, S)
    return _prog_cache[key]


def kernel(hidden_states, gate_w, shared_gate_w, shared_up_w, shared_down_w,
           routed_gate_w, routed_up_w, routed_down_w):
    B, SEQ, Hh = hidden_states.shape
    assert Hh == H
    x = np.ascontiguousarray(hidden_states.reshape(-1, H), dtype=np.float32)
    T = x.shape[0]
    assert T % NCORES == 0
    S = T // NCORES

    # ---- host routing (dispatch) ----
    logits = x @ gate_w.T.astype(np.float32)          # [T, E]
    top_id = logits.argmax(-1)
    top_val = logits.max(-1)
    scale = 1.0 / (1.0 + np.exp(-top_val))
    order = np.argsort(top_id, kind="stable")
    counts = np.bincount(top_id, minlength=E)
    starts = np.zeros(E + 1, np.int64)
    starts[1:] = np.cumsum(counts)
    cap = max(512, int(-(-counts.max() // 128)) * 128)

    sorted_x = x[order]                                # [T, H] unscaled
    sorted_xs = sorted_x * scale[order][:, None]       # [T, H] scaled

    sgT = np.ascontiguousarray(shared_gate_w.T.astype(np.float32))   # [H, I]
    suT = np.ascontiguousarray(shared_up_w.T.astype(np.float32))     # [H, I]
    sdT = np.ascontiguousarray(shared_down_w.T.astype(np.float32))   # [I, H]
    gwT = np.ascontiguousarray(gate_w.T.astype(np.float32))          # [H, E]

    in_maps = []
    for c in range(NCORES):
        n_c = int(counts[c])
        xr = np.zeros((H, cap), np.float32)
        xr[:, :n_c] = sorted_xs[starts[c]:starts[c + 1]].T
        xsT = np.ascontiguousarray(sorted_x[c * S:(c + 1) * S].T)
        in_maps.append({
            "xr": xr,
            "xs": xsT,
            "wg": np.ascontiguousarray(routed_gate_w[c].T.astype(np.float32)),
            "wu": np.ascontiguousarray(routed_up_w[c].T.astype(np.float32)),
            "wd": np.ascontiguousarray(routed_down_w[c].T.astype(np.float32)),
            "sg": sgT,
            "su": suT,
            "sd": sdT,
            "gwt": gwT,
        })

    nc = _get_program(cap, S)
    res = run_bass_kernel_spmd(nc, in_maps, core_ids=list(range(NCORES)))

    # ---- host combine ----
    routed_all = np.concatenate(
        [res.results[e]["outr"][:counts[e]] for e in range(E)], axis=0)
    shared_all = np.concatenate(
        [res.results[c]["outs"] for c in range(NCORES)], axis=0)
    out_flat = np.empty((T, H), np.float32)
    out_flat[order] = routed_all + shared_all

    logt_sorted = np.concatenate(
        [res.results[c]["logt"].T for c in range(NCORES)], axis=0)  # [T, E]
    rl = np.empty((T, E), np.float32)
    rl[order] = logt_sorted

    return out_flat.reshape(B, SEQ, H), rl.reshape(B, SEQ, E)


# revision 4
# speedup vs baseline: 1.4647x; 1.4647x over previous
"""Llama4-style MoE (top-1 routing, E=8) + shared SwiGLU expert on 8 Trainium2 cores.

Strategy (expert-parallel + data-parallel shared, host dispatch/combine):
  - Host computes router logits / top-1 routing / sigmoid scaling (0.03% of FLOPs)
    and sorts tokens by expert — the "dispatch" step of the sharding.
  - Core e gets: the tokens routed to expert e (zero-padded to a uniform CAP),
    expert e's SwiGLU weights, plus a 1/8 slice of all tokens and the replicated
    shared-expert weights.  Each core runs two SwiGLU MLPs (routed segment +
    shared segment) and the router-logits matmul for its shared slice.
  - All matmuls in float32r (fp32 storage, ~bf16 rate on the PE at free>=256).
  - Host packs every device input into the exact SBUF tile layout so each DMA
    is one contiguous >=8KB-per-partition read (DMA packet efficiency).
  - Host "combine": routed and shared partial outputs are summed and scattered
    back to the original token order.

Device layouts (P=128 partitions):
  xt   [P, 16, M]          x^T tokens:      xt[p, hc, m]  = x[m, hc*128+p]
  w1   [16, P, 16, P]      gate/up weights: w1[ig, p, hc, i] = W[ig*128+i, hc*128+p]
  w2   [16, P, 16, P]      down weights:    w2[hp, pi, ic, hj] = Wd[hp*128+hj, ic*128+pi]
  out  [H, M]              output^T (host transposes back)
"""

import numpy as np
from contextlib import ExitStack

import concourse.bacc as bacc
import concourse.tile as tile
from concourse import mybir
from concourse.bass_utils import run_bass_kernel_spmd

P = 128
H = 2048
I = 2048
E = 8
NCORES = 8
HC = H // P    # 16 contraction chunks (stage 1)
IC = I // P    # 16 i chunks

f32 = mybir.dt.float32
f32r = mybir.dt.float32r

_prog_cache: dict = {}


def _m_tiles(M, width=512):
    out = []
    m0 = 0
    while m0 < M:
        w = min(width, M - m0)
        out.append((m0, w))
        m0 += w
    return out


def _build_program(cap, S):
    nc = bacc.Bacc("TRN2", target_bir_lowering=False, debug=False,
                   num_devices=NCORES)

    xr = nc.declare_dram_parameter("xr", [P, HC, cap], f32r, isOutput=False)
    xs = nc.declare_dram_parameter("xs", [P, HC, S], f32r, isOutput=False)
    wg = nc.declare_dram_parameter("wg", [IC, P, HC, P], f32r, isOutput=False)
    wu = nc.declare_dram_parameter("wu", [IC, P, HC, P], f32r, isOutput=False)
    wd = nc.declare_dram_parameter("wd", [HC, P, IC, P], f32r, isOutput=False)
    sg = nc.declare_dram_parameter("sg", [IC, P, HC, P], f32r, isOutput=False)
    su = nc.declare_dram_parameter("su", [IC, P, HC, P], f32r, isOutput=False)
    sd = nc.declare_dram_parameter("sd", [HC, P, IC, P], f32r, isOutput=False)
    gwt = nc.declare_dram_parameter("gwt", [P, HC, E], f32r, isOutput=False)
    outr = nc.declare_dram_parameter("outr", [H, cap], f32, isOutput=True)
    outs = nc.declare_dram_parameter("outs", [H, S], f32, isOutput=True)
    logt = nc.declare_dram_parameter("logt", [E, S], f32, isOutput=True)

    with tile.TileContext(nc) as tc, ExitStack() as ctx:
        xpool = ctx.enter_context(tc.tile_pool(name="xp", bufs=1))
        w1pool = ctx.enter_context(tc.tile_pool(name="w1", bufs=2))
        hpool = ctx.enter_context(tc.tile_pool(name="hp", bufs=1))
        w2pool = ctx.enter_context(tc.tile_pool(name="w2", bufs=3))
        tpool = ctx.enter_context(tc.tile_pool(name="tp", bufs=3))
        opool = ctx.enter_context(tc.tile_pool(name="op", bufs=3))
        ps1 = ctx.enter_context(tc.tile_pool(name="ps1", bufs=2, space="PSUM"))
        ps2 = ctx.enter_context(tc.tile_pool(name="ps2", bufs=3, space="PSUM"))

        segs = [
            ("r", xr, cap, wg, wu, wd, outr),
            ("s", xs, S, sg, su, sd, outs),
        ]

        for sname, xdram, M, wgd, wud, wdd, odram in segs:
            mts = _m_tiles(M)
            xt = xpool.tile([P, HC, M], f32r, name=f"xt_{sname}", tag="xt")
            nc.sync.dma_start(out=xt, in_=xdram.ap())

            ht = hpool.tile([P, IC, M], f32r, name=f"ht_{sname}",
                            tag=f"ht_{sname}")

            # ---- stage 1: g = Wg^T x, u = Wu^T x, ht = silu(g) * u ----
            for ig in range(IC):
                wgt = w1pool.tile([P, HC, P], f32r, name="wgt", tag="wgt")
                wut = w1pool.tile([P, HC, P], f32r, name="wut", tag="wut")
                nc.sync.dma_start(out=wgt, in_=wgd.ap()[ig])
                nc.sync.dma_start(out=wut, in_=wud.ap()[ig])
                for (m0, mw) in mts:
                    pg = ps1.tile([P, 512], f32, name="pg", tag="pg")
                    pu = ps1.tile([P, 512], f32, name="pu", tag="pu")
                    for hc in range(HC):
                        nc.tensor.matmul(pg[:, :mw], lhsT=wgt[:, hc, :],
                                         rhs=xt[:, hc, m0:m0 + mw],
                                         start=(hc == 0), stop=(hc == HC - 1))
                    for hc in range(HC):
                        nc.tensor.matmul(pu[:, :mw], lhsT=wut[:, hc, :],
                                         rhs=xt[:, hc, m0:m0 + mw],
                                         start=(hc == 0), stop=(hc == HC - 1))
                    sil = tpool.tile([P, 512], f32, name="sil", tag="sil")
                    nc.scalar.activation(sil[:, :mw], pg[:, :mw],
                                         mybir.ActivationFunctionType.Silu)
                    nc.vector.tensor_mul(ht[:, ig, m0:m0 + mw], sil[:, :mw],
                                         pu[:, :mw])

            # ---- stage 2: out^T[hj, m] = sum_i Wd[hj, i] * ht[i, m] ----
            for hp in range(HC):
                wdt = w2pool.tile([P, IC, P], f32r, name="wdt", tag="wdt")
                nc.sync.dma_start(out=wdt, in_=wdd.ap()[hp])
                for (m0, mw) in mts:
                    pd = ps2.tile([P, 512], f32, name="pd", tag="pd")
                    for ic in range(IC):
                        nc.tensor.matmul(pd[:, :mw], lhsT=wdt[:, ic, :],
                                         rhs=ht[:, ic, m0:m0 + mw],
                                         start=(ic == 0), stop=(ic == IC - 1))
                    ot = opool.tile([P, 512], f32, name="ot", tag="ot")
                    nc.vector.tensor_copy(ot[:, :mw], pd[:, :mw])
                    nc.sync.dma_start(
                        out=odram.ap()[hp * P:(hp + 1) * P, m0:m0 + mw],
                        in_=ot[:, :mw])

            if sname == "s":
                # ---- router logits for this core's shared token slice ----
                gwtile = xpool.tile([P, HC, E], f32r, name="gwtile",
                                    tag="gwtile")
                nc.sync.dma_start(out=gwtile, in_=gwt.ap())
                for (m0, mw) in mts:
                    pl = ps2.tile([E, 512], f32, name="pl", tag="pd")
                    for hc in range(HC):
                        nc.tensor.matmul(pl[:, :mw], lhsT=gwtile[:, hc, :],
                                         rhs=xt[:, hc, m0:m0 + mw],
                                         start=(hc == 0), stop=(hc == HC - 1))
                    lt = opool.tile([E, 512], f32, name="lt", tag="lt")
                    nc.vector.tensor_copy(lt[:, :mw], pl[:, :mw])
                    nc.sync.dma_start(out=logt.ap()[:, m0:m0 + mw],
                                      in_=lt[:, :mw])

    nc.compile()
    return nc


def _get_program(cap, S):
    key = (cap, S)
    if key not in _prog_cache:
        _prog_cache[key] = _build_program(cap, S)
    return _prog_cache[key]


def _pack_x(seg_x):
    # [M, H] -> [P, HC, M]; [p, hc, m] = x[m, hc*128+p]
    M = seg_x.shape[0]
    return np.ascontiguousarray(seg_x.reshape(M, HC, P).transpose(2, 1, 0))


def _pack_w1(w):
    # [I, H] -> [IC, P(p=h sub), HC, P(i)]; [ig, p, hc, i] = w[ig*128+i, hc*128+p]
    return np.ascontiguousarray(
        w.reshape(IC, P, HC, P).transpose(0, 3, 2, 1))


def _pack_w2(wd_):
    # [H, I] -> [HC, P(pi=i sub), IC, P(hj)]; [hp, pi, ic, hj] = wd[hp*128+hj, ic*128+pi]
    return np.ascontiguousarray(
        wd_.reshape(HC, P, IC, P).transpose(0, 3, 2, 1))


def kernel(hidden_states, gate_w, shared_gate_w, shared_up_w, shared_down_w,
           routed_gate_w, routed_up_w, routed_down_w):
    B, SEQ, Hh = hidden_states.shape
    assert Hh == H
    x = np.ascontiguousarray(hidden_states.reshape(-1, H), dtype=np.float32)
    T = x.shape[0]
    assert T % NCORES == 0
    S = T // NCORES

    # ---- host routing (dispatch) ----
    logits = x @ gate_w.T.astype(np.float32)          # [T, E]
    top_id = logits.argmax(-1)
    top_val = logits.max(-1)
    scale = 1.0 / (1.0 + np.exp(-top_val))
    order = np.argsort(top_id, kind="stable")
    counts = np.bincount(top_id, minlength=E)
    starts = np.zeros(E + 1, np.int64)
    starts[1:] = np.cumsum(counts)
    cap = max(512, int(-(-counts.max() // 128)) * 128)

    sorted_x = x[order]                                # [T, H] unscaled
    sorted_xs = sorted_x * scale[order][:, None]       # [T, H] scaled

    sgP = _pack_w1(np.asarray(shared_gate_w, np.float32))
    suP = _pack_w1(np.asarray(shared_up_w, np.float32))
    sdP = _pack_w2(np.asarray(shared_down_w, np.float32))
    gwP = np.ascontiguousarray(
        np.asarray(gate_w, np.float32).reshape(E, HC, P).transpose(2, 1, 0))

    in_maps = []
    for c in range(NCORES):
        n_c = int(counts[c])
        seg = np.zeros((cap, H), np.float32)
        seg[:n_c] = sorted_xs[starts[c]:starts[c + 1]]
        in_maps.append({
            "xr": _pack_x(seg),
            "xs": _pack_x(sorted_x[c * S:(c + 1) * S]),
            "wg": _pack_w1(np.asarray(routed_gate_w[c], np.float32)),
            "wu": _pack_w1(np.asarray(routed_up_w[c], np.float32)),
            "wd": _pack_w2(np.asarray(routed_down_w[c], np.float32)),
            "sg": sgP,
            "su": suP,
            "sd": sdP,
            "gwt": gwP,
        })

    nc = _get_program(cap, S)
    res = run_bass_kernel_spmd(nc, in_maps, core_ids=list(range(NCORES)))

    # ---- host combine ----
    routed_all = np.concatenate(
        [res.results[e]["outr"].T[:counts[e]] for e in range(E)], axis=0)
    shared_all = np.concatenate(
        [res.results[c]["outs"].T for c in range(NCORES)], axis=0)
    out_flat = np.empty((T, H), np.float32)
    out_flat[order] = routed_all + shared_all

    logt_sorted = np.concatenate(
        [res.results[c]["logt"].T for c in range(NCORES)], axis=0)  # [T, E]
    rl = np.empty((T, E), np.float32)
    rl[order] = logt_sorted

    return out_flat.reshape(B, SEQ, H), rl.reshape(B, SEQ, E)
